# revision 13
# baseline (speedup 1.0000x reference)
"""DecayTemporalGraphNetwork Trainium2 kernel (8 NeuronCores, SPMD).

Contract: kernel(**inputs) takes the FULL unsharded inputs (numpy arrays, keys
as in reference.setup_inputs()) and returns the full outputs
(link_probs [B,1], src_score [B], dst_score [B]).

Strategy
--------
The updated memory table is NOT an output; only rows touched by this batch
(<= 2*B unique node ids) ever influence the outputs.  The host does *integer
index* work only: resolves duplicate-id "last occurrence wins" winners,
compacts the touched rows of the [1M,128] memory table into a [2B,129] table
(col 128 = last_update), and computes forwarding indices.  All floating-point
math and all gathers of model state run on the NeuronCores.

Device-side, data-parallel over the edge batch (2048 edges/core):
  stage A: gather memory rows for src ids, decay, GRU update -> new_src
  AllGather(new_src)                  (cross-edge forwarding, rank-major)
  stage B: gather rows for dst ids, overwrite with forwarded new_src rows
           where the dst id was updated by a src edge, decay, GRU -> new_dst
  AllGather(new_src ++ new_dst)
  embeddings: gather final winner rows, attention decay^2, node MLP, proj MLP
  link predictor -> sigmoid probabilities

Matmul inputs / forwarded state use bf16 (fp32 PSUM accumulation); the decay
/ score path (exp of timestamp deltas) and the memory table stay fp32.
"""

import numpy as np

NCORES = 8
B = 16384
E = B // NCORES          # 2048 edges per core
NUM_NODES = 1_000_000
MEM = 128
NF = 172                 # node/edge feature dim
DECAY = 0.1
P = 128
T = E // P               # 16 row-tiles of 128 edges
CH = 512                 # matmul free-dim chunk
NCH = E // CH            # 4
TPC = CH // P            # tiles per chunk = 4
NU = 2 * B               # compact table capacity (unique ids <= 2B)
TW = MEM + 1             # table width: 128 memory + last_update

_BASS_CACHE = None

# bias-matrix column indices
B_EDGE, B_MP1, B_MP2, B_R, B_Z, B_IN, B_HN, B_NODE, B_P1, B_P2, B_L1, B_L2 = range(12)


def _build_bass():
    import concourse.bass as bass
    import concourse.bacc as bacc
    import concourse.mybir as mybir
    import concourse.tile as tile
    from concourse.masks import make_identity

    f32 = mybir.dt.float32
    bf16 = mybir.dt.bfloat16
    i32 = mybir.dt.int32
    AF = mybir.ActivationFunctionType
    OP = mybir.AluOpType
    IOA = bass.IndirectOffsetOnAxis

    nc = bacc.Bacc("TRN2", target_bir_lowering=False, debug=False, num_devices=NCORES)

    # ---------------- I/O ----------------
    tbl = nc.dram_tensor("tbl", [NU, TW], f32, kind="ExternalInput")
    eftT = nc.dram_tensor("eftT", [NF, E], bf16, kind="ExternalInput")
    sftT = nc.dram_tensor("sftT", [NF, E], bf16, kind="ExternalInput")
    dftT = nc.dram_tensor("dftT", [NF, E], bf16, kind="ExternalInput")
    ipack_d = nc.dram_tensor("ipack", [P, 5 * T], i32, kind="ExternalInput")
    fpack_d = nc.dram_tensor("fpack", [P, 5 * T], f32, kind="ExternalInput")
    wpA_d = nc.dram_tensor("wpA", [P, 1921], bf16, kind="ExternalInput")
    wpB_d = nc.dram_tensor("wpB", [NF - P, 256], bf16, kind="ExternalInput")
    biasm_d = nc.dram_tensor("biasm", [P, 12], f32, kind="ExternalInput")
    probs_o = nc.dram_tensor("probs_o", [1, E], f32, kind="ExternalOutput")
    ssc_o = nc.dram_tensor("ssc_o", [P, T], f32, kind="ExternalOutput")
    dsc_o = nc.dram_tensor("dsc_o", [P, T], f32, kind="ExternalOutput")

    with tile.TileContext(nc) as tc:
        with (
            tc.tile_pool(name="wp", bufs=1) as wp,
            tc.tile_pool(name="persist", bufs=1) as pp,
            tc.tile_pool(name="work", bufs=3) as wk,
            tc.tile_pool(name="gth", bufs=6) as gp,
            tc.tile_pool(name="rm", bufs=4) as rp,
            tc.tile_pool(name="psA", bufs=4, space="PSUM") as psA,
            tc.tile_pool(name="psT", bufs=3, space="PSUM") as psT,
            tc.tile_pool(name="psL", bufs=1, space="PSUM") as psL,
            tc.tile_pool(name="dram", bufs=1, space="DRAM") as dp,
        ):
            # ------------- packed loads (few big DMAs) -------------
            ipack = wp.tile([P, 5 * T], i32)
            nc.sync.dma_start(ipack[:], ipack_d[:, :])
            srcpos, dstpos, fwd, gsrc, gdst = (
                ipack[:, k * T:(k + 1) * T] for k in range(5))
            wpA = wp.tile([P, 1921], bf16)
            nc.sync.dma_start(wpA[:], wpA_d[:, :])
            wpB = wp.tile([NF - P, 256], bf16)
            nc.sync.dma_start(wpB[:], wpB_d[:, :])

            def wA(c0, c1):
                return wpA[:, c0:c1]
            w_edge0, w_node0 = wA(0, 128), wA(128, 256)
            w_mp1, w_mp2 = wA(256, 384), wA(384, 512)
            w_ihT, w_hhT = wA(512, 896), wA(896, 1280)
            w_p1a, w_p1b = wA(1280, 1408), wA(1408, 1536)
            w_p2 = wA(1536, 1664)
            w_l1a, w_l1b = wA(1664, 1792), wA(1792, 1920)
            w_l2 = wA(1920, 1921)
            w_edge1, w_node1 = wpB[:, 0:128], wpB[:, 128:256]

            ef0 = pp.tile([P, E], bf16)
            nc.sync.dma_start(ef0[:], eftT[:P, :])
            ef1 = pp.tile([NF - P, E], bf16)
            nc.sync.dma_start(ef1[:], eftT[P:NF, :])
            fpack = wp.tile([P, 5 * T], f32)
            nc.sync.dma_start(fpack[:], fpack_d[:, :])
            tsv, hitv, lubv, lu2s, lu2d = (
                fpack[:, k * T:(k + 1) * T] for k in range(5))
            biasm = wp.tile([P, 12], f32)
            nc.sync.dma_start(biasm[:], biasm_d[:, :])
            ident = wp.tile([P, P], bf16)
            make_identity(nc, ident[:])
            sf0 = pp.tile([P, E], bf16)
            nc.sync.dma_start(sf0[:], sftT[:P, :])
            sf1 = pp.tile([NF - P, E], bf16)
            nc.sync.dma_start(sf1[:], sftT[P:NF, :])
            df0 = pp.tile([P, E], bf16)
            nc.sync.dma_start(df0[:], dftT[:P, :])
            df1 = pp.tile([NF - P, E], bf16)
            nc.sync.dma_start(df1[:], dftT[P:NF, :])

            def bias(col):
                return biasm[:, col:col + 1]

            # persistent activations (feature-major [128, E], bf16)
            proc = pp.tile([P, E], bf16)
            neS = pp.tile([P, E], bf16)
            neD = pp.tile([P, E], bf16)
            giB_r = pp.tile([P, E], bf16)
            giB_z = pp.tile([P, E], bf16)
            giB_n = pp.tile([P, E], bf16)
            sembT = pp.tile([P, E], bf16)
            dembT = pp.tile([P, E], bf16)
            probs_sb = pp.tile([1, E], f32)

            # DRAM scratch + collective buffers (bf16).  cat rows 0:E new_src,
            # E:2E new_dst.  1024-row halves are AllGather'd as they complete
            # (contiguous Shared outs), then strided D2D DMAs assemble
            # all_comb[rank*2E + (0:E ns | E:2E nd)] == one big AllGather of
            # cat, but pipelined with compute.
            cat = dp.tile([2 * E, MEM], bf16)
            ns_ag = dp.tile([B, MEM], bf16, addr_space="Shared")
            nd_ag = dp.tile([B, MEM], bf16, addr_space="Shared")
            all_comb = dp.tile([2 * B, MEM], bf16)
            acv = all_comb[:].rearrange("(r p) d -> r p d", p=2 * E)

            # ---------------- helpers ----------------
            def copyback(idx, dst_ap, src_ap):
                if idx % 2 == 0:
                    nc.vector.tensor_copy(dst_ap, src_ap)
                else:
                    nc.scalar.activation(dst_ap, src_ap, AF.Copy)

            def addbias(c, out_ap, ps, col):
                if c % 2 == 0:
                    nc.vector.tensor_scalar_add(out_ap, ps[:], bias(col))
                else:
                    nc.scalar.activation(out_ap, ps[:], AF.Identity, bias=bias(col))

            def linear_feat(x0, x1, w0, w1, bias_col, out_tile, act, c):
                cs = slice(c * CH, (c + 1) * CH)
                ps = psA.tile([P, CH], f32, space="PSUM", tag="psA")
                nc.tensor.matmul(ps[:], lhsT=w0, rhs=x0[:, cs], start=True, stop=False)
                nc.tensor.matmul(ps[:], lhsT=w1, rhs=x1[:, cs], start=False, stop=True)
                if act is None:
                    addbias(c, out_tile[:, cs], ps, bias_col)
                else:
                    nc.scalar.activation(out_tile[:, cs], ps[:], act, bias=bias(bias_col))

            def mm1(wtile, rhs_ap, tag="psA"):
                ps = psA.tile([P, CH], f32, space="PSUM", tag=tag)
                nc.tensor.matmul(ps[:], lhsT=wtile, rhs=rhs_ap, start=True, stop=True)
                return ps

            def gru_chunk_fused(curdT_c, c, newT_c):
                cs = slice(c * CH, (c + 1) * CH)
                x = proc[:, cs]
                ps_r = psA.tile([P, CH], f32, space="PSUM", tag="psA")
                nc.tensor.matmul(ps_r[:], lhsT=w_ihT[:, 0:P], rhs=x, start=True, stop=False)
                nc.tensor.matmul(ps_r[:], lhsT=w_hhT[:, 0:P], rhs=curdT_c[:], start=False, stop=True)
                r = wk.tile([P, CH], bf16, tag="r")
                nc.scalar.activation(r[:], ps_r[:], AF.Sigmoid, bias=bias(B_R))
                ps_z = psA.tile([P, CH], f32, space="PSUM", tag="psA")
                nc.tensor.matmul(ps_z[:], lhsT=w_ihT[:, P:2 * P], rhs=x, start=True, stop=False)
                nc.tensor.matmul(ps_z[:], lhsT=w_hhT[:, P:2 * P], rhs=curdT_c[:], start=False, stop=True)
                z = wk.tile([P, CH], bf16, tag="z")
                nc.scalar.activation(z[:], ps_z[:], AF.Sigmoid, bias=bias(B_Z))
                ps_hn = psA.tile([P, CH], f32, space="PSUM", tag="psA")
                nc.tensor.matmul(ps_hn[:], lhsT=w_hhT[:, 2 * P:3 * P], rhs=curdT_c[:], start=True, stop=True)
                hn = wk.tile([P, CH], bf16, tag="hn")
                nc.vector.tensor_scalar_add(hn[:], ps_hn[:], bias(B_HN))
                ps_in = psA.tile([P, CH], f32, space="PSUM", tag="psA")
                nc.tensor.matmul(ps_in[:], lhsT=w_ihT[:, 2 * P:3 * P], rhs=x, start=True, stop=True)
                tmp = wk.tile([P, CH], bf16, tag="tmp")
                nc.vector.tensor_tensor(tmp[:], r[:], hn[:], OP.mult)
                t2 = wk.tile([P, CH], f32, tag="t2")
                nc.vector.tensor_tensor(t2[:], ps_in[:], tmp[:], OP.add)
                n = wk.tile([P, CH], bf16, tag="n")
                nc.scalar.activation(n[:], t2[:], AF.Tanh, bias=bias(B_IN))
                d = wk.tile([P, CH], bf16, tag="d")
                nc.vector.tensor_tensor(d[:], curdT_c[:], n[:], OP.subtract)
                e = wk.tile([P, CH], bf16, tag="e")
                nc.vector.tensor_tensor(e[:], z[:], d[:], OP.mult)
                nc.vector.tensor_tensor(newT_c[:], n[:], e[:], OP.add)

            def gru_chunk_pregi(curdT_c, c, newT_c):
                cs = slice(c * CH, (c + 1) * CH)
                ps_r = mm1(w_hhT[:, 0:P], curdT_c[:])
                rr = wk.tile([P, CH], f32, tag="t2")
                nc.vector.tensor_tensor(rr[:], ps_r[:], giB_r[:, cs], OP.add)
                r = wk.tile([P, CH], bf16, tag="r")
                nc.scalar.activation(r[:], rr[:], AF.Sigmoid, bias=bias(B_R))
                ps_z = mm1(w_hhT[:, P:2 * P], curdT_c[:])
                zz = wk.tile([P, CH], f32, tag="t2")
                nc.vector.tensor_tensor(zz[:], ps_z[:], giB_z[:, cs], OP.add)
                z = wk.tile([P, CH], bf16, tag="z")
                nc.scalar.activation(z[:], zz[:], AF.Sigmoid, bias=bias(B_Z))
                ps_hn = mm1(w_hhT[:, 2 * P:3 * P], curdT_c[:])
                hn = wk.tile([P, CH], bf16, tag="hn")
                nc.vector.tensor_scalar_add(hn[:], ps_hn[:], bias(B_HN))
                tmp = wk.tile([P, CH], bf16, tag="tmp")
                nc.vector.tensor_tensor(tmp[:], r[:], hn[:], OP.mult)
                t2 = wk.tile([P, CH], f32, tag="t2")
                nc.vector.tensor_tensor(t2[:], giB_n[:, cs], tmp[:], OP.add)
                n = wk.tile([P, CH], bf16, tag="n")
                nc.scalar.activation(n[:], t2[:], AF.Tanh, bias=bias(B_IN))
                d = wk.tile([P, CH], bf16, tag="d")
                nc.vector.tensor_tensor(d[:], curdT_c[:], n[:], OP.subtract)
                e = wk.tile([P, CH], bf16, tag="e")
                nc.vector.tensor_tensor(e[:], z[:], d[:], OP.mult)
                nc.vector.tensor_tensor(newT_c[:], n[:], e[:], OP.add)

            def write_rows(newT_c, c, row0):
                for i in range(TPC):
                    t = c * TPC + i
                    pst = psT.tile([P, P], bf16, space="PSUM", tag="psT")
                    nc.tensor.transpose(pst[:], newT_c[:, i * P:(i + 1) * P], ident[:])
                    rm = rp.tile([P, P], bf16, tag="rm")
                    copyback(t, rm[:], pst[:])
                    nc.sync.dma_start(cat[row0 + t * P: row0 + (t + 1) * P, :], rm[:])

            # ---------------- message MLP (proc) ----------------
            ee = pp.tile([P, E], bf16)
            for c in range(NCH):
                linear_feat(ef0, ef1, w_edge0, w_edge1, B_EDGE, ee, None, c)
            for c in range(NCH):
                cs = slice(c * CH, (c + 1) * CH)
                ps1 = mm1(w_mp1, ee[:, cs])
                h1 = wk.tile([P, CH], bf16, tag="h1")
                nc.scalar.activation(h1[:], ps1[:], AF.Relu, bias=bias(B_MP1))
                ps2 = mm1(w_mp2, h1[:])
                addbias(c, proc[:, cs], ps2, B_MP2)

            # ---------------- gathers ----------------
            dtA = wp.tile([P, T], f32)
            decA = wp.tile([P, T], f32)
            gA_tiles = []
            for t in range(T):
                gA = gp.tile([P, TW], f32, tag="gA", bufs=8)
                nc.gpsimd.indirect_dma_start(
                    out=gA[:], out_offset=None, in_=tbl[:, :],
                    in_offset=IOA(ap=srcpos[:, t:t + 1], axis=0))
                nc.vector.tensor_tensor(dtA[:, t:t + 1], tsv[:, t:t + 1],
                                        gA[:, MEM:MEM + 1], OP.subtract)
                gA_tiles.append(gA)
            dtB = wp.tile([P, T], f32)
            decB = wp.tile([P, T], f32)
            lue = wp.tile([P, T], f32)
            luB0 = wp.tile([P, T], f32)
            gB_tiles = []
            for t in range(T):
                gB = gp.tile([P, TW], f32, tag="gB", bufs=16)
                nc.gpsimd.indirect_dma_start(
                    out=gB[:], out_offset=None, in_=tbl[:, :],
                    in_offset=IOA(ap=dstpos[:, t:t + 1], axis=0))
                nc.vector.tensor_copy(luB0[:, t:t + 1], gB[:, MEM:MEM + 1])
                gB_tiles.append(gB)
            nc.vector.tensor_tensor(lue[:], lubv[:], luB0[:], OP.subtract)
            nc.vector.tensor_tensor(lue[:], lue[:], hitv[:], OP.mult)
            nc.vector.tensor_tensor(lue[:], lue[:], luB0[:], OP.add)
            nc.vector.tensor_tensor(dtB[:], tsv[:], lue[:], OP.subtract)
            nc.vector.tensor_scalar_max(dtB[:], dtB[:], 0.0)
            nc.scalar.activation(decB[:], dtB[:], AF.Exp, scale=-DECAY)

            # ---------------- stage A ----------------
            for c in range(NCH):
                chs = slice(c * TPC, (c + 1) * TPC)
                nc.vector.tensor_scalar_max(dtA[:, chs], dtA[:, chs], 0.0)
                nc.scalar.activation(decA[:, chs], dtA[:, chs], AF.Exp, scale=-DECAY)
                curdT_c = wk.tile([P, CH], bf16, tag="curdT")
                for i in range(TPC):
                    t = c * TPC + i
                    curd = rp.tile([P, MEM], bf16, tag="curd")
                    nc.vector.tensor_scalar_mul(curd[:], gA_tiles[t][:, 0:MEM],
                                                decA[:, t:t + 1])
                    pst = psT.tile([P, P], bf16, space="PSUM", tag="psT")
                    nc.tensor.transpose(pst[:], curd[:], ident[:])
                    copyback(t, curdT_c[:, i * P:(i + 1) * P], pst[:])
                newT_c = wk.tile([P, CH], bf16, tag="newT")
                gru_chunk_fused(curdT_c, c, newT_c)
                write_rows(newT_c, c, 0)

            # stage-B gi preacts (independent of AG; fills the bubble)
            for c in range(NCH):
                cs = slice(c * CH, (c + 1) * CH)
                ps = mm1(w_ihT[:, 0:P], proc[:, cs])
                nc.vector.tensor_copy(giB_r[:, cs], ps[:])
                ps = mm1(w_ihT[:, P:2 * P], proc[:, cs])
                nc.vector.tensor_copy(giB_z[:, cs], ps[:])
                ps = mm1(w_ihT[:, 2 * P:3 * P], proc[:, cs])
                nc.vector.tensor_copy(giB_n[:, cs], ps[:])

            nc.gpsimd.collective_compute(
                "AllGather", OP.bypass,
                replica_groups=[list(range(NCORES))],
                ins=[cat[0:E, :]],
                outs=[ns_ag.opt()],
            )

            for c in range(NCH):
                linear_feat(sf0, sf1, w_node0, w_node1, B_NODE, neS, None, c)
            for c in range(NCH):
                linear_feat(df0, df1, w_node0, w_node1, B_NODE, neD, None, c)

            # ---------------- stage B ----------------
            for c in range(NCH):
                curdT_c = wk.tile([P, CH], bf16, tag="curdT")
                for i in range(TPC):
                    t = c * TPC + i
                    gB = gB_tiles[t]
                    fw = gp.tile([P, MEM], bf16, tag="fw", bufs=6)
                    nc.gpsimd.indirect_dma_start(
                        out=fw[:], out_offset=None, in_=cat[:, :],
                        in_offset=IOA(ap=fwd[:, t:t + 1], axis=0))
                    df = rp.tile([P, MEM], f32, tag="df", bufs=6)
                    nc.vector.tensor_tensor(df[:], fw[:], gB[:, 0:MEM], OP.subtract)
                    nc.vector.tensor_scalar_mul(df[:], df[:], hitv[:, t:t + 1])
                    curd = rp.tile([P, MEM], f32, tag="curdB")
                    nc.vector.tensor_tensor(curd[:], gB[:, 0:MEM], df[:], OP.add)
                    curdb = rp.tile([P, MEM], bf16, tag="curdBb")
                    nc.vector.tensor_scalar_mul(curdb[:], curd[:], decB[:, t:t + 1])
                    pst = psT.tile([P, P], bf16, space="PSUM", tag="psT")
                    nc.tensor.transpose(pst[:], curdb[:], ident[:])
                    copyback(t, curdT_c[:, i * P:(i + 1) * P], pst[:])
                newT_c = wk.tile([P, CH], bf16, tag="newT")
                gru_chunk_pregi(curdT_c, c, newT_c)
                write_rows(newT_c, c, E)
            nc.gpsimd.dma_start(
                acv[:, 0:E, :], ns_ag[:].rearrange("(r p) d -> r p d", p=E))
            nc.gpsimd.collective_compute(
                "AllGather", OP.bypass,
                replica_groups=[list(range(NCORES))],
                ins=[cat[E:2 * E, :]],
                outs=[nd_ag.opt()],
            )
            nc.gpsimd.dma_start(
                acv[:, E:2 * E, :], nd_ag[:].rearrange("(r p) d -> r p d", p=E))

            # ---------------- embeddings ----------------
            def emb_side(gidx, lu2, ne, embT, score_out):
                dt2 = wp.tile([P, T], f32, name="dt2", uniquify=True)
                score = wp.tile([P, T], f32, name="score", uniquify=True)
                s2 = wp.tile([P, T], f32, name="s2", uniquify=True)
                nc.vector.tensor_tensor(dt2[:], tsv[:], lu2[:], OP.subtract)
                nc.vector.tensor_scalar_max(dt2[:], dt2[:], 0.0)
                nc.scalar.activation(score[:], dt2[:], AF.Exp, scale=-DECAY)
                nc.sync.dma_start(score_out[:, :], score[:])
                nc.vector.tensor_tensor(s2[:], score[:], score[:], OP.mult)
                for c in range(NCH):
                    attT_c = wk.tile([P, CH], bf16, tag="attT")
                    for i in range(TPC):
                        t = c * TPC + i
                        gm = gp.tile([P, MEM], bf16, tag="gm")
                        nc.gpsimd.indirect_dma_start(
                            out=gm[:], out_offset=None, in_=all_comb[:, :],
                            in_offset=IOA(ap=gidx[:, t:t + 1], axis=0))
                        att = rp.tile([P, MEM], bf16, tag="att")
                        nc.vector.tensor_scalar_mul(att[:], gm[:], s2[:, t:t + 1])
                        pst = psT.tile([P, P], bf16, space="PSUM", tag="psT")
                        nc.tensor.transpose(pst[:], att[:], ident[:])
                        copyback(t, attT_c[:, i * P:(i + 1) * P], pst[:])
                    cs = slice(c * CH, (c + 1) * CH)
                    ps1 = psA.tile([P, CH], f32, space="PSUM", tag="psA")
                    nc.tensor.matmul(ps1[:], lhsT=w_p1a, rhs=attT_c[:], start=True, stop=False)
                    nc.tensor.matmul(ps1[:], lhsT=w_p1b, rhs=ne[:, cs], start=False, stop=True)
                    h = wk.tile([P, CH], bf16, tag="h")
                    nc.scalar.activation(h[:], ps1[:], AF.Relu, bias=bias(B_P1))
                    ps2 = mm1(w_p2, h[:])
                    addbias(c, embT[:, cs], ps2, B_P2)

            emb_side(gsrc, lu2s, neS, sembT, ssc_o)
            emb_side(gdst, lu2d, neD, dembT, dsc_o)

            # ---------------- link predictor ----------------
            for c in range(NCH):
                cs = slice(c * CH, (c + 1) * CH)
                psl = psA.tile([P, CH], f32, space="PSUM", tag="psA")
                nc.tensor.matmul(psl[:], lhsT=w_l1a, rhs=sembT[:, cs], start=True, stop=False)
                nc.tensor.matmul(psl[:], lhsT=w_l1b, rhs=dembT[:, cs], start=False, stop=True)
                hl = wk.tile([P, CH], bf16, tag="hl")
                nc.scalar.activation(hl[:], psl[:], AF.Relu, bias=bias(B_L1))
                pso = psL.tile([1, CH], f32, space="PSUM", tag="psL")
                nc.tensor.matmul(pso[:], lhsT=w_l2, rhs=hl[:], start=True, stop=True)
                nc.scalar.activation(probs_sb[0:1, cs], pso[:], AF.Sigmoid,
                                     bias=biasm[0:1, B_L2:B_L2 + 1])
            nc.sync.dma_start(probs_o[:, :], probs_sb[:])

    nc.compile()
    return nc


def _get_bass():
    global _BASS_CACHE
    if _BASS_CACHE is None:
        _BASS_CACHE = _build_bass()
    return _BASS_CACHE


def _host_prepare(inputs):
    import ml_dtypes
    bf = ml_dtypes.bfloat16

    src = np.clip(np.asarray(inputs["src_ids"]).astype(np.int64).ravel(), 0, NUM_NODES - 1)
    dst = np.clip(np.asarray(inputs["dst_ids"]).astype(np.int64).ravel(), 0, NUM_NODES - 1)
    ts = np.asarray(inputs["timestamps"], dtype=np.float32).ravel()
    mem0 = np.asarray(inputs["memory0"], dtype=np.float32)
    lu0 = np.asarray(inputs["last_update0"], dtype=np.float32)

    pos = np.arange(B, dtype=np.int64)
    last_src = np.full(NUM_NODES, -1, np.int64)
    last_src[src] = pos
    last_dst = np.full(NUM_NODES, -1, np.int64)
    last_dst[dst] = pos

    uniq = np.unique(np.concatenate([src, dst]))
    U = uniq.size
    cpos = np.zeros(NUM_NODES, np.int32)
    cpos[uniq] = np.arange(U, dtype=np.int32)
    tbl = np.zeros((NU, TW), np.float32)
    tbl[:U, :MEM] = mem0[uniq]
    tbl[:U, MEM] = lu0[uniq]

    ks_full = last_src[dst]                  # winner src occurrence of each dst id
    hit = ks_full >= 0

    # ---- edge permutation: co-locate each hit dst edge with its src winner,
    # so stage-B forwarding reads the LOCAL new_src buffer (no AllGather). ----
    parent = {}

    def find(x):
        r = x
        while parent.get(r, r) != r:
            r = parent[r]
        while parent.get(x, x) != x:
            parent[x], x = r, parent[x]
        return r

    hit_idx = np.nonzero(hit)[0]
    verts = set()
    for i in hit_idx:
        a, b = int(i), int(ks_full[i])
        verts.add(a)
        verts.add(b)
        ra, rb = find(a), find(b)
        if ra != rb:
            parent[ra] = rb
    groups = {}
    for v in verts:
        groups.setdefault(find(v), []).append(v)
    constrained = np.zeros(B, bool)
    for v in verts:
        constrained[v] = True
    slot_of = np.arange(B, dtype=np.int64)     # edge -> slot
    occupant = np.arange(B, dtype=np.int64)    # slot -> edge
    free_slots = [[s for s in range(c * E, (c + 1) * E) if not constrained[s]]
                  for c in range(NCORES)]
    for g in groups.values():
        if len(g) < 2:
            continue
        target = int(slot_of[g[0]]) // E
        for m in g[1:]:
            if slot_of[m] // E == target:
                continue
            s = free_slots[target].pop()
            f = occupant[s]
            sm = slot_of[m]
            occupant[s], occupant[sm] = m, f
            slot_of[m], slot_of[f] = s, sm
            free_slots[sm // E].append(int(sm))
    order = occupant                           # slot -> original edge

    # slot-ordered per-edge data
    src_o, dst_o, ts_o = src[order], dst[order], ts[order]
    src_pos = cpos[src_o]
    dst_pos = cpos[dst_o]
    hit_o = hit[order]
    ksc = np.where(hit_o, ks_full[order], 0)
    assert np.all((slot_of[ksc] // E)[hit_o] == (np.arange(B) // E)[hit_o])
    fwd = (slot_of[ksc] % E).astype(np.int32)  # LOCAL cat row (new_src half)
    lub = np.where(hit_o, ts[ksc], 0.0).astype(np.float32)
    hitf = hit_o.astype(np.float32)

    def final_ref(ids):
        j = last_dst[ids]
        k = last_src[ids]
        used = j >= 0
        jj = np.where(used, j, 0)
        kk = np.where(k >= 0, k, 0)
        pj = slot_of[jj]
        pk = slot_of[kk]
        row_d = (pj // E) * (2 * E) + E + (pj % E)
        row_s = (pk // E) * (2 * E) + (pk % E)
        rows = np.where(used, row_d, row_s).astype(np.int32)
        lu2 = np.where(used, ts[jj], ts[kk]).astype(np.float32)
        return rows, lu2

    gsrc, lu2s = final_ref(src_o)
    gdst, lu2d = final_ref(dst_o)

    wi = {k: np.asarray(inputs[k], dtype=np.float32) for k in
          ["W_edge", "b_edge", "W_mp1", "b_mp1", "W_mp2", "b_mp2", "W_ih", "W_hh",
           "b_ih", "b_hh", "W_node", "b_node", "W_p1", "b_p1", "W_p2", "b_p2",
           "W_l1", "b_l1", "W_l2", "b_l2"]}
    biasm = np.zeros((P, 12), np.float32)
    biasm[:, B_EDGE] = wi["b_edge"]
    biasm[:, B_MP1] = wi["b_mp1"]
    biasm[:, B_MP2] = wi["b_mp2"]
    biasm[:, B_R] = wi["b_ih"][0:MEM] + wi["b_hh"][0:MEM]
    biasm[:, B_Z] = wi["b_ih"][MEM:2 * MEM] + wi["b_hh"][MEM:2 * MEM]
    biasm[:, B_IN] = wi["b_ih"][2 * MEM:3 * MEM]
    biasm[:, B_HN] = wi["b_hh"][2 * MEM:3 * MEM]
    biasm[:, B_NODE] = wi["b_node"]
    biasm[:, B_P1] = wi["b_p1"]
    biasm[:, B_P2] = wi["b_p2"]
    biasm[:, B_L1] = wi["b_l1"]
    biasm[0, B_L2] = wi["b_l2"][0]

    # pack weights: [128, 1921] (K<=128 parts) and [44, 256] (K=172 tails)
    wpA = np.zeros((P, 1921), np.float32)
    wpA[:, 0:128] = wi["W_edge"][:P]
    wpA[:, 128:256] = wi["W_node"][:P]
    wpA[:, 256:384] = wi["W_mp1"]
    wpA[:, 384:512] = wi["W_mp2"]
    wpA[:, 512:896] = wi["W_ih"].T
    wpA[:, 896:1280] = wi["W_hh"].T
    wpA[:, 1280:1408] = wi["W_p1"][:P]
    wpA[:, 1408:1536] = wi["W_p1"][P:]
    wpA[:, 1536:1664] = wi["W_p2"]
    wpA[:, 1664:1792] = wi["W_l1"][:P]
    wpA[:, 1792:1920] = wi["W_l1"][P:]
    wpA[:, 1920:1921] = wi["W_l2"]
    wpB = np.zeros((NF - P, 256), np.float32)
    wpB[:, 0:128] = wi["W_edge"][P:]
    wpB[:, 128:256] = wi["W_node"][P:]

    shared = dict(
        tbl=tbl,
        wpA=wpA.astype(bf),
        wpB=wpB.astype(bf),
        biasm=biasm,
    )

    ef = np.asarray(inputs["edge_features"], dtype=np.float32)
    sf = np.asarray(inputs["src_features"], dtype=np.float32)
    df_ = np.asarray(inputs["dst_features"], dtype=np.float32)

    def pt(v):
        return np.ascontiguousarray(v.reshape(T, P).T)

    in_maps = []
    for c in range(NCORES):
        sl = slice(c * E, (c + 1) * E)
        m = dict(shared)
        ipack = np.concatenate(
            [pt(v[sl]) for v in (src_pos, dst_pos, fwd, gsrc, gdst)], axis=1)
        fpack = np.concatenate(
            [pt(v[sl]) for v in (ts_o, hitf, lub, lu2s, lu2d)], axis=1)
        osl = order[sl]
        m.update(
            eftT=np.ascontiguousarray(ef[osl].T).astype(bf),
            sftT=np.ascontiguousarray(sf[osl].T).astype(bf),
            dftT=np.ascontiguousarray(df_[osl].T).astype(bf),
            ipack=np.ascontiguousarray(ipack.astype(np.int32)),
            fpack=np.ascontiguousarray(fpack.astype(np.float32)),
        )
        in_maps.append(m)
    return in_maps, order


def _run(inputs, trace=False):
    from concourse.bass_utils import run_bass_kernel_spmd

    nc = _get_bass()
    in_maps, order = _host_prepare(inputs)
    res = run_bass_kernel_spmd(nc, in_maps, core_ids=list(range(NCORES)), trace=trace)

    probs_s = np.concatenate([res.results[c]["probs_o"][0] for c in range(NCORES)])
    ssc_s = np.concatenate([res.results[c]["ssc_o"].T.reshape(E) for c in range(NCORES)])
    dsc_s = np.concatenate([res.results[c]["dsc_o"].T.reshape(E) for c in range(NCORES)])
    probs = np.empty(B, np.float32)
    ssc = np.empty(B, np.float32)
    dsc = np.empty(B, np.float32)
    probs[order] = probs_s
    ssc[order] = ssc_s
    dsc[order] = dsc_s
    out = (probs.reshape(B, 1), ssc, dsc)
    return out, res


def kernel(**inputs):
    out, _ = _run(inputs, trace=False)
    return out


# revision 15
# speedup vs baseline: 1.2675x; 1.2675x over previous
"""DecayTemporalGraphNetwork Trainium2 kernel (8 NeuronCores, SPMD).

Contract: kernel(**inputs) takes the FULL unsharded inputs (numpy arrays, keys
as in reference.setup_inputs()) and returns the full outputs
(link_probs [B,1], src_score [B], dst_score [B]).

Strategy
--------
The updated memory table is NOT an output; only rows touched by this batch
(<= 2*B unique node ids) ever influence the outputs.  The host does *integer
index* work only: resolves duplicate-id "last occurrence wins" winners,
compacts the touched rows of the [1M,128] memory table into a [2B,129] table
(col 128 = last_update), and computes forwarding indices.  All floating-point
math and all gathers of model state run on the NeuronCores.

Device-side, data-parallel over the edge batch (2048 edges/core):
  stage A: gather memory rows for src ids, decay, GRU update -> new_src
  AllGather(new_src)                  (cross-edge forwarding, rank-major)
  stage B: gather rows for dst ids, overwrite with forwarded new_src rows
           where the dst id was updated by a src edge, decay, GRU -> new_dst
  AllGather(new_src ++ new_dst)
  embeddings: gather final winner rows, attention decay^2, node MLP, proj MLP
  link predictor -> sigmoid probabilities

Matmul inputs / forwarded state use bf16 (fp32 PSUM accumulation); the decay
/ score path (exp of timestamp deltas) and the memory table stay fp32.
"""

import numpy as np

NCORES = 8
B = 16384
E = B // NCORES          # 2048 edges per core
NUM_NODES = 1_000_000
MEM = 128
NF = 172                 # node/edge feature dim
DECAY = 0.1
P = 128
T = E // P               # 16 row-tiles of 128 edges
CH = 512                 # matmul free-dim chunk
NCH = E // CH            # 4
TPC = CH // P            # tiles per chunk = 4
NU = 2 * B               # compact table capacity (unique ids <= 2B)
TW = MEM + 1             # table width: 128 memory + last_update

_BASS_CACHE = None

# bias-matrix column indices
B_EDGE, B_MP1, B_MP2, B_R, B_Z, B_IN, B_HN, B_NODE, B_P1, B_P2, B_L1, B_L2 = range(12)


def _build_bass():
    import concourse.bass as bass
    import concourse.bacc as bacc
    import concourse.mybir as mybir
    import concourse.tile as tile
    from concourse.masks import make_identity

    f32 = mybir.dt.float32
    bf16 = mybir.dt.bfloat16
    i32 = mybir.dt.int32
    AF = mybir.ActivationFunctionType
    OP = mybir.AluOpType
    IOA = bass.IndirectOffsetOnAxis

    nc = bacc.Bacc("TRN2", target_bir_lowering=False, debug=False, num_devices=NCORES)

    # ---------------- I/O ----------------
    tbl = nc.dram_tensor("tbl", [NU, TW], f32, kind="ExternalInput")
    eftT = nc.dram_tensor("eftT", [NF, E], bf16, kind="ExternalInput")
    sftT = nc.dram_tensor("sftT", [NF, E], bf16, kind="ExternalInput")
    dftT = nc.dram_tensor("dftT", [NF, E], bf16, kind="ExternalInput")
    ipack_d = nc.dram_tensor("ipack", [P, 5 * T], i32, kind="ExternalInput")
    fpack_d = nc.dram_tensor("fpack", [P, 5 * T], f32, kind="ExternalInput")
    wpA_d = nc.dram_tensor("wpA", [P, 1921], bf16, kind="ExternalInput")
    wpB_d = nc.dram_tensor("wpB", [NF - P, 256], bf16, kind="ExternalInput")
    biasm_d = nc.dram_tensor("biasm", [P, 12], f32, kind="ExternalInput")
    probs_o = nc.dram_tensor("probs_o", [1, E], f32, kind="ExternalOutput")
    ssc_o = nc.dram_tensor("ssc_o", [P, T], f32, kind="ExternalOutput")
    dsc_o = nc.dram_tensor("dsc_o", [P, T], f32, kind="ExternalOutput")

    with tile.TileContext(nc) as tc:
        with (
            tc.tile_pool(name="wp", bufs=1) as wp,
            tc.tile_pool(name="persist", bufs=1) as pp,
            tc.tile_pool(name="work", bufs=3) as wk,
            tc.tile_pool(name="gth", bufs=6) as gp,
            tc.tile_pool(name="rm", bufs=4) as rp,
            tc.tile_pool(name="psA", bufs=4, space="PSUM") as psA,
            tc.tile_pool(name="psT", bufs=3, space="PSUM") as psT,
            tc.tile_pool(name="psL", bufs=1, space="PSUM") as psL,
            tc.tile_pool(name="dram", bufs=1, space="DRAM") as dp,
        ):
            # ------------- packed loads (few big DMAs) -------------
            ipack = wp.tile([P, 5 * T], i32)
            nc.sync.dma_start(ipack[:], ipack_d[:, :])
            srcpos, dstpos, fwd, gsrc, gdst = (
                ipack[:, k * T:(k + 1) * T] for k in range(5))
            wpA = wp.tile([P, 1921], bf16)
            nc.sync.dma_start(wpA[:], wpA_d[:, :])
            wpB = wp.tile([NF - P, 256], bf16)
            nc.sync.dma_start(wpB[:], wpB_d[:, :])

            def wA(c0, c1):
                return wpA[:, c0:c1]
            w_edge0, w_node0 = wA(0, 128), wA(128, 256)
            w_mp1, w_mp2 = wA(256, 384), wA(384, 512)
            w_ihT, w_hhT = wA(512, 896), wA(896, 1280)
            w_p1a, w_p1b = wA(1280, 1408), wA(1408, 1536)
            w_p2 = wA(1536, 1664)
            w_l1a, w_l1b = wA(1664, 1792), wA(1792, 1920)
            w_l2 = wA(1920, 1921)
            w_edge1, w_node1 = wpB[:, 0:128], wpB[:, 128:256]

            ef0 = pp.tile([P, E], bf16)
            nc.sync.dma_start(ef0[:], eftT[:P, :])
            ef1 = pp.tile([NF - P, E], bf16)
            nc.sync.dma_start(ef1[:], eftT[P:NF, :])
            fpack = wp.tile([P, 5 * T], f32)
            nc.sync.dma_start(fpack[:], fpack_d[:, :])
            tsv, hitv, lubv, lu2s, lu2d = (
                fpack[:, k * T:(k + 1) * T] for k in range(5))
            biasm = wp.tile([P, 12], f32)
            nc.sync.dma_start(biasm[:], biasm_d[:, :])
            ident = wp.tile([P, P], bf16)
            make_identity(nc, ident[:])
            sf0 = pp.tile([P, E], bf16)
            nc.sync.dma_start(sf0[:], sftT[:P, :])
            sf1 = pp.tile([NF - P, E], bf16)
            nc.sync.dma_start(sf1[:], sftT[P:NF, :])
            df0 = pp.tile([P, E], bf16)
            nc.sync.dma_start(df0[:], dftT[:P, :])
            df1 = pp.tile([NF - P, E], bf16)
            nc.sync.dma_start(df1[:], dftT[P:NF, :])

            def bias(col):
                return biasm[:, col:col + 1]

            # persistent activations (feature-major [128, E], bf16)
            proc = pp.tile([P, E], bf16)
            neS = pp.tile([P, E], bf16)
            neD = pp.tile([P, E], bf16)
            giB_r = pp.tile([P, E], bf16)
            giB_z = pp.tile([P, E], bf16)
            giB_n = pp.tile([P, E], bf16)
            sembT = pp.tile([P, E], bf16)
            dembT = pp.tile([P, E], bf16)
            probs_sb = pp.tile([1, E], f32)

            # DRAM scratch + collective buffers (bf16).  cat rows 0:E new_src,
            # E:2E new_dst.  1024-row halves are AllGather'd as they complete
            # (contiguous Shared outs), then strided D2D DMAs assemble
            # all_comb[rank*2E + (0:E ns | E:2E nd)] == one big AllGather of
            # cat, but pipelined with compute.
            cat = dp.tile([2 * E, MEM], bf16)
            all_comb = dp.tile([2 * B, MEM], bf16, addr_space="Shared")

            # ---------------- helpers ----------------
            def copyback(idx, dst_ap, src_ap):
                if idx % 2 == 0:
                    nc.vector.tensor_copy(dst_ap, src_ap)
                else:
                    nc.scalar.activation(dst_ap, src_ap, AF.Copy)

            def addbias(c, out_ap, ps, col):
                if c % 2 == 0:
                    nc.vector.tensor_scalar_add(out_ap, ps[:], bias(col))
                else:
                    nc.scalar.activation(out_ap, ps[:], AF.Identity, bias=bias(col))

            def linear_feat(x0, x1, w0, w1, bias_col, out_tile, act, c):
                cs = slice(c * CH, (c + 1) * CH)
                ps = psA.tile([P, CH], f32, space="PSUM", tag="psA")
                nc.tensor.matmul(ps[:], lhsT=w0, rhs=x0[:, cs], start=True, stop=False)
                nc.tensor.matmul(ps[:], lhsT=w1, rhs=x1[:, cs], start=False, stop=True)
                if act is None:
                    addbias(c, out_tile[:, cs], ps, bias_col)
                else:
                    nc.scalar.activation(out_tile[:, cs], ps[:], act, bias=bias(bias_col))

            def mm1(wtile, rhs_ap, tag="psA"):
                ps = psA.tile([P, CH], f32, space="PSUM", tag=tag)
                nc.tensor.matmul(ps[:], lhsT=wtile, rhs=rhs_ap, start=True, stop=True)
                return ps

            def gru_chunk_fused(curdT_c, c, newT_c):
                cs = slice(c * CH, (c + 1) * CH)
                x = proc[:, cs]
                ps_r = psA.tile([P, CH], f32, space="PSUM", tag="psA")
                nc.tensor.matmul(ps_r[:], lhsT=w_ihT[:, 0:P], rhs=x, start=True, stop=False)
                nc.tensor.matmul(ps_r[:], lhsT=w_hhT[:, 0:P], rhs=curdT_c[:], start=False, stop=True)
                r = wk.tile([P, CH], bf16, tag="r")
                nc.scalar.activation(r[:], ps_r[:], AF.Sigmoid, bias=bias(B_R))
                ps_z = psA.tile([P, CH], f32, space="PSUM", tag="psA")
                nc.tensor.matmul(ps_z[:], lhsT=w_ihT[:, P:2 * P], rhs=x, start=True, stop=False)
                nc.tensor.matmul(ps_z[:], lhsT=w_hhT[:, P:2 * P], rhs=curdT_c[:], start=False, stop=True)
                z = wk.tile([P, CH], bf16, tag="z")
                nc.scalar.activation(z[:], ps_z[:], AF.Sigmoid, bias=bias(B_Z))
                ps_hn = psA.tile([P, CH], f32, space="PSUM", tag="psA")
                nc.tensor.matmul(ps_hn[:], lhsT=w_hhT[:, 2 * P:3 * P], rhs=curdT_c[:], start=True, stop=True)
                hn = wk.tile([P, CH], bf16, tag="hn")
                nc.vector.tensor_scalar_add(hn[:], ps_hn[:], bias(B_HN))
                ps_in = psA.tile([P, CH], f32, space="PSUM", tag="psA")
                nc.tensor.matmul(ps_in[:], lhsT=w_ihT[:, 2 * P:3 * P], rhs=x, start=True, stop=True)
                tmp = wk.tile([P, CH], bf16, tag="tmp")
                nc.vector.tensor_tensor(tmp[:], r[:], hn[:], OP.mult)
                t2 = wk.tile([P, CH], f32, tag="t2")
                nc.vector.tensor_tensor(t2[:], ps_in[:], tmp[:], OP.add)
                n = wk.tile([P, CH], bf16, tag="n")
                nc.scalar.activation(n[:], t2[:], AF.Tanh, bias=bias(B_IN))
                d = wk.tile([P, CH], bf16, tag="d")
                nc.vector.tensor_tensor(d[:], curdT_c[:], n[:], OP.subtract)
                e = wk.tile([P, CH], bf16, tag="e")
                nc.vector.tensor_tensor(e[:], z[:], d[:], OP.mult)
                nc.vector.tensor_tensor(newT_c[:], n[:], e[:], OP.add)

            def gru_chunk_pregi(curdT_c, c, newT_c):
                cs = slice(c * CH, (c + 1) * CH)
                ps_r = mm1(w_hhT[:, 0:P], curdT_c[:])
                rr = wk.tile([P, CH], f32, tag="t2")
                nc.vector.tensor_tensor(rr[:], ps_r[:], giB_r[:, cs], OP.add)
                r = wk.tile([P, CH], bf16, tag="r")
                nc.scalar.activation(r[:], rr[:], AF.Sigmoid, bias=bias(B_R))
                ps_z = mm1(w_hhT[:, P:2 * P], curdT_c[:])
                zz = wk.tile([P, CH], f32, tag="t2")
                nc.vector.tensor_tensor(zz[:], ps_z[:], giB_z[:, cs], OP.add)
                z = wk.tile([P, CH], bf16, tag="z")
                nc.scalar.activation(z[:], zz[:], AF.Sigmoid, bias=bias(B_Z))
                ps_hn = mm1(w_hhT[:, 2 * P:3 * P], curdT_c[:])
                hn = wk.tile([P, CH], bf16, tag="hn")
                nc.vector.tensor_scalar_add(hn[:], ps_hn[:], bias(B_HN))
                tmp = wk.tile([P, CH], bf16, tag="tmp")
                nc.vector.tensor_tensor(tmp[:], r[:], hn[:], OP.mult)
                t2 = wk.tile([P, CH], f32, tag="t2")
                nc.vector.tensor_tensor(t2[:], giB_n[:, cs], tmp[:], OP.add)
                n = wk.tile([P, CH], bf16, tag="n")
                nc.scalar.activation(n[:], t2[:], AF.Tanh, bias=bias(B_IN))
                d = wk.tile([P, CH], bf16, tag="d")
                nc.vector.tensor_tensor(d[:], curdT_c[:], n[:], OP.subtract)
                e = wk.tile([P, CH], bf16, tag="e")
                nc.vector.tensor_tensor(e[:], z[:], d[:], OP.mult)
                nc.vector.tensor_tensor(newT_c[:], n[:], e[:], OP.add)

            def write_rows(newT_c, c, row0):
                for i in range(TPC):
                    t = c * TPC + i
                    pst = psT.tile([P, P], bf16, space="PSUM", tag="psT")
                    nc.tensor.transpose(pst[:], newT_c[:, i * P:(i + 1) * P], ident[:])
                    rm = rp.tile([P, P], bf16, tag="rm")
                    copyback(t, rm[:], pst[:])
                    nc.sync.dma_start(cat[row0 + t * P: row0 + (t + 1) * P, :], rm[:])

            # ---------------- message MLP (proc) ----------------
            ee = pp.tile([P, E], bf16)
            for c in range(NCH):
                linear_feat(ef0, ef1, w_edge0, w_edge1, B_EDGE, ee, None, c)
            for c in range(NCH):
                cs = slice(c * CH, (c + 1) * CH)
                ps1 = mm1(w_mp1, ee[:, cs])
                h1 = wk.tile([P, CH], bf16, tag="h1")
                nc.scalar.activation(h1[:], ps1[:], AF.Relu, bias=bias(B_MP1))
                ps2 = mm1(w_mp2, h1[:])
                addbias(c, proc[:, cs], ps2, B_MP2)

            # ---------------- gathers ----------------
            dtA = wp.tile([P, T], f32)
            decA = wp.tile([P, T], f32)
            gA_tiles = []
            for t in range(T):
                gA = gp.tile([P, TW], f32, tag="gA", bufs=8)
                nc.gpsimd.indirect_dma_start(
                    out=gA[:], out_offset=None, in_=tbl[:, :],
                    in_offset=IOA(ap=srcpos[:, t:t + 1], axis=0))
                nc.vector.tensor_tensor(dtA[:, t:t + 1], tsv[:, t:t + 1],
                                        gA[:, MEM:MEM + 1], OP.subtract)
                gA_tiles.append(gA)
            dtB = wp.tile([P, T], f32)
            decB = wp.tile([P, T], f32)
            lue = wp.tile([P, T], f32)
            luB0 = wp.tile([P, T], f32)
            gB_tiles = []
            for t in range(T):
                gB = gp.tile([P, TW], f32, tag="gB", bufs=16)
                nc.gpsimd.indirect_dma_start(
                    out=gB[:], out_offset=None, in_=tbl[:, :],
                    in_offset=IOA(ap=dstpos[:, t:t + 1], axis=0))
                nc.vector.tensor_copy(luB0[:, t:t + 1], gB[:, MEM:MEM + 1])
                gB_tiles.append(gB)
            nc.vector.tensor_tensor(lue[:], lubv[:], luB0[:], OP.subtract)
            nc.vector.tensor_tensor(lue[:], lue[:], hitv[:], OP.mult)
            nc.vector.tensor_tensor(lue[:], lue[:], luB0[:], OP.add)
            nc.vector.tensor_tensor(dtB[:], tsv[:], lue[:], OP.subtract)
            nc.vector.tensor_scalar_max(dtB[:], dtB[:], 0.0)
            nc.scalar.activation(decB[:], dtB[:], AF.Exp, scale=-DECAY)

            # ---------------- stage A ----------------
            for c in range(NCH):
                chs = slice(c * TPC, (c + 1) * TPC)
                nc.vector.tensor_scalar_max(dtA[:, chs], dtA[:, chs], 0.0)
                nc.scalar.activation(decA[:, chs], dtA[:, chs], AF.Exp, scale=-DECAY)
                curdT_c = wk.tile([P, CH], bf16, tag="curdT")
                for i in range(TPC):
                    t = c * TPC + i
                    curd = rp.tile([P, MEM], bf16, tag="curd")
                    nc.vector.tensor_scalar_mul(curd[:], gA_tiles[t][:, 0:MEM],
                                                decA[:, t:t + 1])
                    pst = psT.tile([P, P], bf16, space="PSUM", tag="psT")
                    nc.tensor.transpose(pst[:], curd[:], ident[:])
                    copyback(t, curdT_c[:, i * P:(i + 1) * P], pst[:])
                newT_c = wk.tile([P, CH], bf16, tag="newT")
                gru_chunk_fused(curdT_c, c, newT_c)
                write_rows(newT_c, c, 0)

            # stage-B gi preacts (independent of AG; fills the bubble)
            for c in range(NCH):
                cs = slice(c * CH, (c + 1) * CH)
                ps = mm1(w_ihT[:, 0:P], proc[:, cs])
                nc.vector.tensor_copy(giB_r[:, cs], ps[:])
                ps = mm1(w_ihT[:, P:2 * P], proc[:, cs])
                nc.vector.tensor_copy(giB_z[:, cs], ps[:])
                ps = mm1(w_ihT[:, 2 * P:3 * P], proc[:, cs])
                nc.vector.tensor_copy(giB_n[:, cs], ps[:])

            for c in range(NCH):
                linear_feat(sf0, sf1, w_node0, w_node1, B_NODE, neS, None, c)
            for c in range(NCH):
                linear_feat(df0, df1, w_node0, w_node1, B_NODE, neD, None, c)

            # ---------------- stage B ----------------
            for c in range(NCH):
                curdT_c = wk.tile([P, CH], bf16, tag="curdT")
                for i in range(TPC):
                    t = c * TPC + i
                    gB = gB_tiles[t]
                    fw = gp.tile([P, MEM], bf16, tag="fw", bufs=6)
                    nc.gpsimd.indirect_dma_start(
                        out=fw[:], out_offset=None, in_=cat[:, :],
                        in_offset=IOA(ap=fwd[:, t:t + 1], axis=0))
                    df = rp.tile([P, MEM], f32, tag="df", bufs=6)
                    nc.vector.tensor_tensor(df[:], fw[:], gB[:, 0:MEM], OP.subtract)
                    nc.vector.tensor_scalar_mul(df[:], df[:], hitv[:, t:t + 1])
                    curd = rp.tile([P, MEM], f32, tag="curdB")
                    nc.vector.tensor_tensor(curd[:], gB[:, 0:MEM], df[:], OP.add)
                    curdb = rp.tile([P, MEM], bf16, tag="curdBb")
                    nc.vector.tensor_scalar_mul(curdb[:], curd[:], decB[:, t:t + 1])
                    pst = psT.tile([P, P], bf16, space="PSUM", tag="psT")
                    nc.tensor.transpose(pst[:], curdb[:], ident[:])
                    copyback(t, curdT_c[:, i * P:(i + 1) * P], pst[:])
                newT_c = wk.tile([P, CH], bf16, tag="newT")
                gru_chunk_pregi(curdT_c, c, newT_c)
                write_rows(newT_c, c, E)
            nc.gpsimd.collective_compute(
                "AllGather", OP.bypass,
                replica_groups=[list(range(NCORES))],
                ins=[cat.opt()],
                outs=[all_comb.opt()],
            )

            # ---------------- embeddings ----------------
            def emb_side(gidx, lu2, ne, embT, score_out):
                dt2 = wp.tile([P, T], f32, name="dt2", uniquify=True)
                score = wp.tile([P, T], f32, name="score", uniquify=True)
                s2 = wp.tile([P, T], f32, name="s2", uniquify=True)
                nc.vector.tensor_tensor(dt2[:], tsv[:], lu2[:], OP.subtract)
                nc.vector.tensor_scalar_max(dt2[:], dt2[:], 0.0)
                nc.scalar.activation(score[:], dt2[:], AF.Exp, scale=-DECAY)
                nc.sync.dma_start(score_out[:, :], score[:])
                nc.vector.tensor_tensor(s2[:], score[:], score[:], OP.mult)
                for c in range(NCH):
                    attT_c = wk.tile([P, CH], bf16, tag="attT")
                    for i in range(TPC):
                        t = c * TPC + i
                        gm = gp.tile([P, MEM], bf16, tag="gm")
                        nc.gpsimd.indirect_dma_start(
                            out=gm[:], out_offset=None, in_=all_comb[:, :],
                            in_offset=IOA(ap=gidx[:, t:t + 1], axis=0))
                        att = rp.tile([P, MEM], bf16, tag="att")
                        nc.vector.tensor_scalar_mul(att[:], gm[:], s2[:, t:t + 1])
                        pst = psT.tile([P, P], bf16, space="PSUM", tag="psT")
                        nc.tensor.transpose(pst[:], att[:], ident[:])
                        copyback(t, attT_c[:, i * P:(i + 1) * P], pst[:])
                    cs = slice(c * CH, (c + 1) * CH)
                    ps1 = psA.tile([P, CH], f32, space="PSUM", tag="psA")
                    nc.tensor.matmul(ps1[:], lhsT=w_p1a, rhs=attT_c[:], start=True, stop=False)
                    nc.tensor.matmul(ps1[:], lhsT=w_p1b, rhs=ne[:, cs], start=False, stop=True)
                    h = wk.tile([P, CH], bf16, tag="h")
                    nc.scalar.activation(h[:], ps1[:], AF.Relu, bias=bias(B_P1))
                    ps2 = mm1(w_p2, h[:])
                    addbias(c, embT[:, cs], ps2, B_P2)

            emb_side(gsrc, lu2s, neS, sembT, ssc_o)
            emb_side(gdst, lu2d, neD, dembT, dsc_o)

            # ---------------- link predictor ----------------
            for c in range(NCH):
                cs = slice(c * CH, (c + 1) * CH)
                psl = psA.tile([P, CH], f32, space="PSUM", tag="psA")
                nc.tensor.matmul(psl[:], lhsT=w_l1a, rhs=sembT[:, cs], start=True, stop=False)
                nc.tensor.matmul(psl[:], lhsT=w_l1b, rhs=dembT[:, cs], start=False, stop=True)
                hl = wk.tile([P, CH], bf16, tag="hl")
                nc.scalar.activation(hl[:], psl[:], AF.Relu, bias=bias(B_L1))
                pso = psL.tile([1, CH], f32, space="PSUM", tag="psL")
                nc.tensor.matmul(pso[:], lhsT=w_l2, rhs=hl[:], start=True, stop=True)
                nc.scalar.activation(probs_sb[0:1, cs], pso[:], AF.Sigmoid,
                                     bias=biasm[0:1, B_L2:B_L2 + 1])
            nc.sync.dma_start(probs_o[:, :], probs_sb[:])

    nc.compile()
    return nc


def _get_bass():
    global _BASS_CACHE
    if _BASS_CACHE is None:
        _BASS_CACHE = _build_bass()
    return _BASS_CACHE


def _host_prepare(inputs):
    import ml_dtypes
    bf = ml_dtypes.bfloat16

    src = np.clip(np.asarray(inputs["src_ids"]).astype(np.int64).ravel(), 0, NUM_NODES - 1)
    dst = np.clip(np.asarray(inputs["dst_ids"]).astype(np.int64).ravel(), 0, NUM_NODES - 1)
    ts = np.asarray(inputs["timestamps"], dtype=np.float32).ravel()
    mem0 = np.asarray(inputs["memory0"], dtype=np.float32)
    lu0 = np.asarray(inputs["last_update0"], dtype=np.float32)

    pos = np.arange(B, dtype=np.int64)
    last_src = np.full(NUM_NODES, -1, np.int64)
    last_src[src] = pos
    last_dst = np.full(NUM_NODES, -1, np.int64)
    last_dst[dst] = pos

    uniq = np.unique(np.concatenate([src, dst]))
    U = uniq.size
    cpos = np.zeros(NUM_NODES, np.int32)
    cpos[uniq] = np.arange(U, dtype=np.int32)
    tbl = np.zeros((NU, TW), np.float32)
    tbl[:U, :MEM] = mem0[uniq]
    tbl[:U, MEM] = lu0[uniq]

    ks_full = last_src[dst]                  # winner src occurrence of each dst id
    hit = ks_full >= 0

    # ---- edge permutation: co-locate each hit dst edge with its src winner,
    # so stage-B forwarding reads the LOCAL new_src buffer (no AllGather). ----
    parent = {}

    def find(x):
        r = x
        while parent.get(r, r) != r:
            r = parent[r]
        while parent.get(x, x) != x:
            parent[x], x = r, parent[x]
        return r

    hit_idx = np.nonzero(hit)[0]
    verts = set()
    for i in hit_idx:
        a, b = int(i), int(ks_full[i])
        verts.add(a)
        verts.add(b)
        ra, rb = find(a), find(b)
        if ra != rb:
            parent[ra] = rb
    groups = {}
    for v in verts:
        groups.setdefault(find(v), []).append(v)
    constrained = np.zeros(B, bool)
    for v in verts:
        constrained[v] = True
    slot_of = np.arange(B, dtype=np.int64)     # edge -> slot
    occupant = np.arange(B, dtype=np.int64)    # slot -> edge
    free_slots = [[s for s in range(c * E, (c + 1) * E) if not constrained[s]]
                  for c in range(NCORES)]
    for g in groups.values():
        if len(g) < 2:
            continue
        target = int(slot_of[g[0]]) // E
        for m in g[1:]:
            if slot_of[m] // E == target:
                continue
            s = free_slots[target].pop()
            f = occupant[s]
            sm = slot_of[m]
            occupant[s], occupant[sm] = m, f
            slot_of[m], slot_of[f] = s, sm
            free_slots[sm // E].append(int(sm))
    order = occupant                           # slot -> original edge

    # slot-ordered per-edge data
    src_o, dst_o, ts_o = src[order], dst[order], ts[order]
    src_pos = cpos[src_o]
    dst_pos = cpos[dst_o]
    hit_o = hit[order]
    ksc = np.where(hit_o, ks_full[order], 0)
    assert np.all((slot_of[ksc] // E)[hit_o] == (np.arange(B) // E)[hit_o])
    fwd = (slot_of[ksc] % E).astype(np.int32)  # LOCAL cat row (new_src half)
    lub = np.where(hit_o, ts[ksc], 0.0).astype(np.float32)
    hitf = hit_o.astype(np.float32)

    def final_ref(ids):
        j = last_dst[ids]
        k = last_src[ids]
        used = j >= 0
        jj = np.where(used, j, 0)
        kk = np.where(k >= 0, k, 0)
        pj = slot_of[jj]
        pk = slot_of[kk]
        row_d = (pj // E) * (2 * E) + E + (pj % E)
        row_s = (pk // E) * (2 * E) + (pk % E)
        rows = np.where(used, row_d, row_s).astype(np.int32)
        lu2 = np.where(used, ts[jj], ts[kk]).astype(np.float32)
        return rows, lu2

    gsrc, lu2s = final_ref(src_o)
    gdst, lu2d = final_ref(dst_o)

    wi = {k: np.asarray(inputs[k], dtype=np.float32) for k in
          ["W_edge", "b_edge", "W_mp1", "b_mp1", "W_mp2", "b_mp2", "W_ih", "W_hh",
           "b_ih", "b_hh", "W_node", "b_node", "W_p1", "b_p1", "W_p2", "b_p2",
           "W_l1", "b_l1", "W_l2", "b_l2"]}
    biasm = np.zeros((P, 12), np.float32)
    biasm[:, B_EDGE] = wi["b_edge"]
    biasm[:, B_MP1] = wi["b_mp1"]
    biasm[:, B_MP2] = wi["b_mp2"]
    biasm[:, B_R] = wi["b_ih"][0:MEM] + wi["b_hh"][0:MEM]
    biasm[:, B_Z] = wi["b_ih"][MEM:2 * MEM] + wi["b_hh"][MEM:2 * MEM]
    biasm[:, B_IN] = wi["b_ih"][2 * MEM:3 * MEM]
    biasm[:, B_HN] = wi["b_hh"][2 * MEM:3 * MEM]
    biasm[:, B_NODE] = wi["b_node"]
    biasm[:, B_P1] = wi["b_p1"]
    biasm[:, B_P2] = wi["b_p2"]
    biasm[:, B_L1] = wi["b_l1"]
    biasm[0, B_L2] = wi["b_l2"][0]

    # pack weights: [128, 1921] (K<=128 parts) and [44, 256] (K=172 tails)
    wpA = np.zeros((P, 1921), np.float32)
    wpA[:, 0:128] = wi["W_edge"][:P]
    wpA[:, 128:256] = wi["W_node"][:P]
    wpA[:, 256:384] = wi["W_mp1"]
    wpA[:, 384:512] = wi["W_mp2"]
    wpA[:, 512:896] = wi["W_ih"].T
    wpA[:, 896:1280] = wi["W_hh"].T
    wpA[:, 1280:1408] = wi["W_p1"][:P]
    wpA[:, 1408:1536] = wi["W_p1"][P:]
    wpA[:, 1536:1664] = wi["W_p2"]
    wpA[:, 1664:1792] = wi["W_l1"][:P]
    wpA[:, 1792:1920] = wi["W_l1"][P:]
    wpA[:, 1920:1921] = wi["W_l2"]
    wpB = np.zeros((NF - P, 256), np.float32)
    wpB[:, 0:128] = wi["W_edge"][P:]
    wpB[:, 128:256] = wi["W_node"][P:]

    shared = dict(
        tbl=tbl,
        wpA=wpA.astype(bf),
        wpB=wpB.astype(bf),
        biasm=biasm,
    )

    ef = np.asarray(inputs["edge_features"], dtype=np.float32)
    sf = np.asarray(inputs["src_features"], dtype=np.float32)
    df_ = np.asarray(inputs["dst_features"], dtype=np.float32)

    def pt(v):
        return np.ascontiguousarray(v.reshape(T, P).T)

    in_maps = []
    for c in range(NCORES):
        sl = slice(c * E, (c + 1) * E)
        m = dict(shared)
        ipack = np.concatenate(
            [pt(v[sl]) for v in (src_pos, dst_pos, fwd, gsrc, gdst)], axis=1)
        fpack = np.concatenate(
            [pt(v[sl]) for v in (ts_o, hitf, lub, lu2s, lu2d)], axis=1)
        osl = order[sl]
        m.update(
            eftT=np.ascontiguousarray(ef[osl].T).astype(bf),
            sftT=np.ascontiguousarray(sf[osl].T).astype(bf),
            dftT=np.ascontiguousarray(df_[osl].T).astype(bf),
            ipack=np.ascontiguousarray(ipack.astype(np.int32)),
            fpack=np.ascontiguousarray(fpack.astype(np.float32)),
        )
        in_maps.append(m)
    return in_maps, order


def _run(inputs, trace=False):
    from concourse.bass_utils import run_bass_kernel_spmd

    nc = _get_bass()
    in_maps, order = _host_prepare(inputs)
    res = run_bass_kernel_spmd(nc, in_maps, core_ids=list(range(NCORES)), trace=trace)

    probs_s = np.concatenate([res.results[c]["probs_o"][0] for c in range(NCORES)])
    ssc_s = np.concatenate([res.results[c]["ssc_o"].T.reshape(E) for c in range(NCORES)])
    dsc_s = np.concatenate([res.results[c]["dsc_o"].T.reshape(E) for c in range(NCORES)])
    probs = np.empty(B, np.float32)
    ssc = np.empty(B, np.float32)
    dsc = np.empty(B, np.float32)
    probs[order] = probs_s
    ssc[order] = ssc_s
    dsc[order] = dsc_s
    out = (probs.reshape(B, 1), ssc, dsc)
    return out, res


def kernel(**inputs):
    out, _ = _run(inputs, trace=False)
    return out
